# revision 26
# baseline (speedup 1.0000x reference)
"""MLA (multi-head latent attention) Trainium2 Bass kernel.

Problem: nn_MLA_20899310862928 — B=8, S=1024, E=2048, H=16, D=128, latent=512,
RoPE on dims 32:128 of each head (non-interleaved halves), causal softmax.

Strategy: data-parallel over batch — each of the 8 NeuronCores handles one
batch element with the full weight set. All host-side layout transforms
(x pre-transpose, weight tiling, head-dim permutation) happen in numpy inside
kernel(); the device only does matmuls/DVE/ACT work with contiguous DMAs.

Per-core pipeline (bf16 matmuls throughout, f32 PSUM accumulation):
  1. ckv projection, then per head h (software-pipelined): q/k projections
     (+RoPE), v for each head pair, and attention for head h-1 interleaved so
     scalar-engine exp overlaps the next head's PE projection work.
  2. scoresT[k,q] per 128-row k-chunk with causal column pruning; exp on the
     scalar engine writes bf16 E tiles; the diagonal 128x128 block is masked
     in-place by a gpsimd affine_select.
  3. PV with a ones-column appended to v: out[q, 0:128] = sum_k E·v and
     out[q, 128] = sum_k E (the softmax denominator) come from the same PSUM
     accumulation, directly in [q, d] layout — no output transposes, no
     separate row-sum matmuls. Normalization fuses into the PSUM->SBUF copy
     via ACT Copy with a per-partition reciprocal scale.

Head-dim permutation: within each head, q/k dims are reordered to
[rope-even(48) | nope(16) | rope-odd(48) | nope(16)] so RoPE pairs sit at a
+64 partition offset (legal SBUF operand bases are 0/32/64/96 only). The same
permutation is applied to Wq and Wk_up columns host-side; scores are
invariant. v stays in natural order, so out needs no un-permute.
"""
import math
import ml_dtypes
import numpy as np
from contextlib import ExitStack

import concourse.bass as bass
import concourse.mybir as mybir
import concourse.tile as tile
from concourse import bacc
from concourse._compat import with_exitstack
from concourse.bass_utils import run_bass_kernel_spmd

F32 = mybir.dt.float32
BF16 = mybir.dt.bfloat16
MULT = mybir.AluOpType.mult
ADD = mybir.AluOpType.add
SUB = mybir.AluOpType.subtract
EXP = mybir.ActivationFunctionType.Exp
COPY = mybir.ActivationFunctionType.Copy

B, S, E, L, H, D = 8, 1024, 2048, 512, 16, 128
NOPE, ROPE_D = 32, 96
NK = E // 128      # 16 contraction chunks for x-projections
NL = L // 128      # 4 contraction chunks for latent projections
NSC = S // 128     # 8 sequence 128-chunks
VW = 130           # v stride per (chunk, head-half): 128 d + 1 ones + 1 pad
SCALE = 1.0 / math.sqrt(D)
THETA = 10000.0


def _head_perm():
    """Within-head dim permutation: new row r -> original head dim."""
    p = np.zeros(128, dtype=np.int64)
    for r in range(48):
        p[r] = 32 + 2 * r            # rope-even
    for r in range(48, 64):
        p[r] = r - 48                # nope 0..15
    for r in range(64, 112):
        p[r] = 33 + 2 * (r - 64)     # rope-odd
    for r in range(112, 128):
        p[r] = 16 + (r - 112)        # nope 16..31
    return p


def host_tensors(Wq, Wkv_down, Wk_up, Wv_up):
    """Permute + tile all weights into the DMA-contiguous device layouts."""
    hp = _head_perm()
    perm = np.concatenate([h * 128 + hp for h in range(H)])
    Wq_p = Wq[:, perm]
    Wk_p = Wk_up[:, perm]

    # [in, out] -> [out_tile, p(in%128), in_chunk, out_in_tile], contiguous
    wq_t = np.ascontiguousarray(
        Wq_p.reshape(NK, 128, H, 128).transpose(2, 1, 0, 3), ml_dtypes.bfloat16)
    wkv_t = np.ascontiguousarray(
        np.asarray(Wkv_down).reshape(NK, 128, NL, 128).transpose(2, 1, 0, 3),
        ml_dtypes.bfloat16)
    wk_t = np.ascontiguousarray(
        Wk_p.reshape(NL, 128, H, 128).transpose(2, 1, 0, 3), ml_dtypes.bfloat16)
    wv_t = np.ascontiguousarray(
        np.asarray(Wv_up).reshape(NL, 128, H // 4, 512).transpose(2, 1, 0, 3),
        ml_dtypes.bfloat16)

    freqs = 1.0 / THETA ** (np.arange(0, ROPE_D, 2, dtype=np.float32) / ROPE_D)
    emb = np.arange(S, dtype=np.float32)[:, None] * freqs[None, :]  # [S, 48]
    cos48 = np.cos(emb).T.astype(np.float32)  # [48, S]
    sin48 = np.sin(emb).T.astype(np.float32)
    ccos = np.zeros((128, S), dtype=np.float32)
    ssin = np.zeros((128, S), dtype=np.float32)
    ccos[0:48] = cos48
    ccos[64:112] = cos48
    ssin[0:48] = sin48
    ssin[64:112] = sin48
    return (wq_t, wkv_t, wk_t, wv_t,
            ccos.astype(ml_dtypes.bfloat16), ssin.astype(ml_dtypes.bfloat16))


@with_exitstack
def mla_kernel(ctx: ExitStack, tc: tile.TileContext, xt_d, wq_d, wkv_d, wk_d, wv_d,
               ccos_d, ssin_d, out_d):
    nc = tc.nc

    pp_const = ctx.enter_context(tc.tile_pool(name="const", bufs=1))
    pp_x = ctx.enter_context(tc.tile_pool(name="xt", bufs=1))
    pp_ckv = ctx.enter_context(tc.tile_pool(name="ckv", bufs=1))
    pp_qk = ctx.enter_context(tc.tile_pool(name="qk", bufs=4))
    pp_w = ctx.enter_context(tc.tile_pool(name="wst", bufs=3))
    pp_rope = ctx.enter_context(tc.tile_pool(name="rope", bufs=2))
    pp_E = ctx.enter_context(tc.tile_pool(name="et", bufs=2))
    pp_v = ctx.enter_context(tc.tile_pool(name="vp", bufs=2))
    pp_o = ctx.enter_context(tc.tile_pool(name="ob", bufs=6))

    ps_a = ctx.enter_context(tc.tile_pool(name="ps_a", bufs=2, space="PSUM"))
    ps_qk = ctx.enter_context(tc.tile_pool(name="ps_qk", bufs=3, space="PSUM"))
    ps_pv = ctx.enter_context(tc.tile_pool(name="ps_pv", bufs=3, space="PSUM"))

    # --- constants ---
    ccos_t = pp_const.tile([128, S], BF16, tag="cct")
    ssin_t = pp_const.tile([128, S], BF16, tag="sst")

    xT2 = []
    wkv_t = []

    def fetch_x(c):
        xt = pp_x.tile([128, 2, S], BF16, tag=f"x{c}", name=f"x{c}")
        nc.sync.dma_start(
            xt[:], xt_d.ap()[c * 256:(c + 1) * 256, :]
            .rearrange("(c p) s -> p c s", p=128))
        xT2.append(xt)

    def fetch_wkv(m):
        wm = pp_w.tile([128, NK, 128], BF16, tag=f"wkv{m}", bufs=1, name=f"wkv{m}")
        nc.sync.dma_start(wm[:], wkv_d.ap()[m])
        wkv_t.append(wm)

    fetch_x(0)
    fetch_wkv(0)
    for c in range(1, 8):
        fetch_x(c)
    for m in range(1, NL):
        fetch_wkv(m)

    wq_t, wk_t, wv_t = {}, {}, {}

    def fetch_head(h):
        wq_t[h] = pp_w.tile([128, NK, 128], BF16, tag="wq", name=f"wq{h}")
        nc.sync.dma_start(wq_t[h][:], wq_d.ap()[h])
        wk_t[h] = pp_w.tile([128, NL, 128], BF16, tag="wk", name=f"wk{h}")
        nc.sync.dma_start(wk_t[h][:], wk_d.ap()[h])
        if h % 4 == 0:
            wv_t[h // 4] = pp_w.tile([128, NL, 512], BF16, tag="wv", bufs=2,
                                     name=f"wv{h // 4}")
            nc.sync.dma_start(wv_t[h // 4][:], wv_d.ap()[h // 4])

    fetch_head(0)
    nc.sync.dma_start(ccos_t[:], ccos_d.ap())
    nc.sync.dma_start(ssin_t[:], ssin_d.ap())

    def rope(t, eng, sfx):
        """In-place RoPE on a [128, S] head tile: rows [E(0:48)|n|O(64:112)|n]."""
        pc = pp_rope.tile([128, S], BF16, tag=f"pc{sfx}", name=f"pc{sfx}")
        pn = pp_rope.tile([128, S], BF16, tag=f"pn{sfx}", name=f"pn{sfx}")
        eng.tensor_tensor(pc[:], t[:], ccos_t[:], MULT)
        eng.tensor_tensor(pn[0:48, :], t[64:112, :], ssin_t[64:112, :], MULT)
        eng.tensor_tensor(pn[64:112, :], t[0:48, :], ssin_t[0:48, :], MULT)
        eng.tensor_tensor(t[0:48, :], pc[0:48, :], pn[0:48, :], SUB)
        eng.tensor_tensor(t[64:112, :], pc[64:112, :], pn[64:112, :], ADD)

    # --- ckv projection: c_kvT [latent, S] in 4 chunks ---
    ckv = [pp_ckv.tile([128, S], BF16, tag=f"ckv{j}", name=f"ckv{j}")
           for j in range(NL)]
    for m in range(NL):
        for n in range(2):
            ps = ps_a.tile([128, 512], F32, tag="pa")
            for k in range(NK):
                nc.tensor.matmul(ps[:], wkv_t[m][:, k],
                                 xT2[k // 2][:, k % 2, n * 512:(n + 1) * 512],
                                 start=(k == 0), stop=(k == NK - 1))
            nc.vector.tensor_copy(ckv[m][:, n * 512:(n + 1) * 512], ps[:])

    qT_t, kT_t, vt_t = {}, {}, {}

    def proj_k(h):
        kt = pp_qk.tile([128, S], BF16, tag="kt", name=f"kt{h}")
        kT_t[h] = kt
        for n in range(2):
            ps = ps_a.tile([128, 512], F32, tag="pa")
            for k in range(NL):
                nc.tensor.matmul(ps[:], wk_t[h][:, k],
                                 ckv[k][:, n * 512:(n + 1) * 512],
                                 start=(k == 0), stop=(k == NL - 1))
            nc.vector.tensor_copy(kt[:, n * 512:(n + 1) * 512], ps[:])
        rope(kt, nc.vector, "g")

    def proj_qv(h):
        qt = pp_qk.tile([128, S], BF16, tag="qt", name=f"qt{h}")
        qT_t[h] = qt
        for n in range(2):
            ps = ps_a.tile([128, 512], F32, tag="pa")
            for k in range(NK):
                nc.tensor.matmul(ps[:], wq_t[h][:, k],
                                 xT2[k // 2][:, k % 2, n * 512:(n + 1) * 512],
                                 start=(k == 0), stop=(k == NK - 1))
            nc.vector.tensor_copy(qt[:, n * 512:(n + 1) * 512], ps[:])
        rope(qt, nc.vector, "v")

        if h % 4 == 0:
            vt = pp_v.tile([128, NSC, 4, VW], BF16, tag="v", name=f"v{h // 4}")
            vt_t[h // 4] = vt
            nc.gpsimd.memset(vt[:, :, :, 128:129], 1.0)
            for sc in range(NSC):
                ps = ps_a.tile([128, 512], F32, tag="pa")
                for k in range(NL):
                    nc.tensor.matmul(ps[:],
                                     ckv[k][:, sc * 128:(sc + 1) * 128],
                                     wv_t[h // 4][:, k],
                                     start=(k == 0), stop=(k == NL - 1))
                nc.scalar.copy(
                    vt[:, sc, :, 0:128],
                    ps[:].rearrange("p (c d) -> p c d", c=4))

    def attn(h):
        hp, hs = h // 4, h % 4
        qt, kt, vt = qT_t[h], kT_t[h], vt_t[hp]
        # Et[kc] holds exp(scores) rows for k-chunk kc, cols q = c0..S
        Et = [pp_E.tile([128, S - 128 * kc], BF16, tag=f"e{kc}", name=f"e{kc}")
              for kc in range(NSC)]
        def scores(kc):
            c0 = 128 * kc
            for n in range(2):
                lo = max(n * 512, c0)
                hi = (n + 1) * 512
                if lo >= hi:
                    continue
                ps = ps_qk.tile([128, 512], F32, tag="qk")
                nc.tensor.matmul(ps[:, lo - 512 * n:512],
                                 kt[:, c0:c0 + 128], qt[:, lo:hi],
                                 start=True, stop=True)
                nc.scalar.activation(Et[kc][:, lo - c0:hi - c0],
                                     ps[:, lo - 512 * n:512], EXP, scale=SCALE)
                if lo == c0:
                    # causal mask of the diagonal block: zero where k > q
                    nc.gpsimd.affine_select(
                        out=Et[kc][:, 0:128], in_=Et[kc][:, 0:128],
                        compare_op=mybir.AluOpType.is_ge, fill=0.0,
                        base=0, pattern=[[1, 128]], channel_multiplier=-1)

        def pv(qc):
            ps = ps_pv.tile([128, 132], F32, tag="pv")
            for kc in range(qc + 1):
                nc.tensor.matmul(ps[:, 0:129],
                                 Et[kc][:, qc * 128 - kc * 128:(qc + 1) * 128 - kc * 128],
                                 vt[:, kc, hs, 0:129],
                                 start=(kc == 0), stop=(kc == qc))
            r = pp_o.tile([128, 1], F32, tag="r")
            nc.vector.reciprocal(r[:], ps[:, 128:129])
            ot = pp_o.tile([128, 128], F32, tag="ot")
            nc.vector.tensor_scalar(ot[:], ps[:, 0:128], r[:, 0:1], None, MULT)
            nc.sync.dma_start(
                out_d.ap()[qc * 128:(qc + 1) * 128, h * 128:(h + 1) * 128],
                ot[:])

        for kc in range(4):
            scores(kc)
        for kc in range(4, NSC):
            pv(kc - 4)
            scores(kc)
        for qc in range(4, NSC):
            pv(qc)

    for h in range(H):
        if h + 1 < H:
            fetch_head(h + 1)
        proj_k(h)
        proj_qv(h)
        if h >= 1:
            attn(h - 1)
    attn(H - 1)


_CACHE = {}


def _build_nc(repeat=1):
    key = ("nc", repeat)
    if key in _CACHE:
        return _CACHE[key]
    nc = bacc.Bacc("TRN2", target_bir_lowering=False, debug=False, num_devices=B)
    xt_d = nc.dram_tensor("xt", [E, S], BF16, kind="ExternalInput")
    wq_d = nc.dram_tensor("wq", [H, 128, NK, 128], BF16, kind="ExternalInput")
    wkv_d = nc.dram_tensor("wkv", [NL, 128, NK, 128], BF16, kind="ExternalInput")
    wk_d = nc.dram_tensor("wk", [H, 128, NL, 128], BF16, kind="ExternalInput")
    wv_d = nc.dram_tensor("wv", [H // 4, 128, NL, 512], BF16, kind="ExternalInput")
    ccos_d = nc.dram_tensor("ccos", [128, S], BF16, kind="ExternalInput")
    ssin_d = nc.dram_tensor("ssin", [128, S], BF16, kind="ExternalInput")
    out_d = nc.dram_tensor("out", [S, E], F32, kind="ExternalOutput")

    with tile.TileContext(nc) as tc:
        for _ in range(repeat):
            mla_kernel(tc, xt_d, wq_d, wkv_d, wk_d, wv_d, ccos_d, ssin_d, out_d)
    nc.compile()
    _CACHE[key] = nc
    return nc


def kernel(x, Wq, Wkv_down, Wk_up, Wv_up, **run_kwargs):
    x = np.asarray(x, dtype=np.float32)
    wq_t, wkv_t, wk_t, wv_t, ccos, ssin = host_tensors(
        np.asarray(Wq, np.float32), np.asarray(Wkv_down, np.float32),
        np.asarray(Wk_up, np.float32), np.asarray(Wv_up, np.float32))
    nc = _build_nc()
    in_maps = [
        {"xt": np.ascontiguousarray(x[b].T.astype(ml_dtypes.bfloat16)),
         "wq": wq_t, "wkv": wkv_t, "wk": wk_t, "wv": wv_t,
         "ccos": ccos, "ssin": ssin}
        for b in range(B)
    ]
    res = run_bass_kernel_spmd(nc, in_maps, core_ids=list(range(B)), **run_kwargs)
    out = np.stack([res.results[b]["out"] for b in range(B)], axis=0)
    if run_kwargs:
        _CACHE["last_res"] = res
    return out


# revision 27
# speedup vs baseline: 1.1899x; 1.1899x over previous
"""MLA (multi-head latent attention) Trainium2 Bass kernel.

Problem: nn_MLA_20899310862928 — B=8, S=1024, E=2048, H=16, D=128, latent=512,
RoPE on dims 32:128 of each head (non-interleaved halves), causal softmax.

Strategy: data-parallel over batch — each of the 8 NeuronCores handles one
batch element with the full weight set. All host-side layout transforms
(x pre-transpose, weight tiling, head-dim permutation) happen in numpy inside
kernel(); the device only does matmuls/DVE/ACT work with contiguous DMAs.

Per-core pipeline (bf16 matmuls throughout, f32 PSUM accumulation):
  1. ckv projection, then per head h (software-pipelined): q/k projections
     (+RoPE), v for each head pair, and attention for head h-1 interleaved so
     scalar-engine exp overlaps the next head's PE projection work.
  2. scoresT[k,q] per 128-row k-chunk with causal column pruning; exp on the
     scalar engine writes bf16 E tiles; the diagonal 128x128 block is masked
     in-place by a gpsimd affine_select.
  3. PV with a ones-column appended to v: out[q, 0:128] = sum_k E·v and
     out[q, 128] = sum_k E (the softmax denominator) come from the same PSUM
     accumulation, directly in [q, d] layout — no output transposes, no
     separate row-sum matmuls. Normalization fuses into the PSUM->SBUF copy
     via ACT Copy with a per-partition reciprocal scale.

Head-dim permutation: within each head, q/k dims are reordered to
[rope-even(48) | nope(16) | rope-odd(48) | nope(16)] so RoPE pairs sit at a
+64 partition offset (legal SBUF operand bases are 0/32/64/96 only). The same
permutation is applied to Wq and Wk_up columns host-side; scores are
invariant. v stays in natural order, so out needs no un-permute.
"""
import math
import ml_dtypes
import numpy as np
from contextlib import ExitStack

import concourse.bass as bass
import concourse.mybir as mybir
import concourse.tile as tile
from concourse import bacc
from concourse._compat import with_exitstack
from concourse.bass_utils import run_bass_kernel_spmd

F32 = mybir.dt.float32
BF16 = mybir.dt.bfloat16
MULT = mybir.AluOpType.mult
ADD = mybir.AluOpType.add
SUB = mybir.AluOpType.subtract
EXP = mybir.ActivationFunctionType.Exp
COPY = mybir.ActivationFunctionType.Copy

B, S, E, L, H, D = 8, 1024, 2048, 512, 16, 128
NOPE, ROPE_D = 32, 96
NK = E // 128      # 16 contraction chunks for x-projections
NL = L // 128      # 4 contraction chunks for latent projections
NSC = S // 128     # 8 sequence 128-chunks
VW = 130           # v stride per (chunk, head-half): 128 d + 1 ones + 1 pad
SCALE = 1.0 / math.sqrt(D)
THETA = 10000.0


def _head_perm():
    """Within-head dim permutation: new row r -> original head dim."""
    p = np.zeros(128, dtype=np.int64)
    for r in range(48):
        p[r] = 32 + 2 * r            # rope-even
    for r in range(48, 64):
        p[r] = r - 48                # nope 0..15
    for r in range(64, 112):
        p[r] = 33 + 2 * (r - 64)     # rope-odd
    for r in range(112, 128):
        p[r] = 16 + (r - 112)        # nope 16..31
    return p


def host_tensors(Wq, Wkv_down, Wk_up, Wv_up):
    """Permute + tile all weights into the DMA-contiguous device layouts."""
    hp = _head_perm()
    perm = np.concatenate([h * 128 + hp for h in range(H)])
    Wq_p = Wq[:, perm]
    Wk_p = Wk_up[:, perm]

    # [in, out] -> [out_tile, p(in%128), in_chunk, out_in_tile], contiguous
    wq_t = np.ascontiguousarray(
        Wq_p.reshape(NK, 128, H, 128).transpose(2, 1, 0, 3), ml_dtypes.bfloat16)
    wkv_t = np.ascontiguousarray(
        np.asarray(Wkv_down).reshape(NK, 128, NL, 128).transpose(2, 1, 0, 3),
        ml_dtypes.bfloat16)
    wk_t = np.ascontiguousarray(
        Wk_p.reshape(NL, 128, H, 128).transpose(2, 1, 0, 3), ml_dtypes.bfloat16)
    wv_t = np.ascontiguousarray(
        np.asarray(Wv_up).reshape(NL, 128, H // 4, 512).transpose(2, 1, 0, 3),
        ml_dtypes.bfloat16)

    freqs = 1.0 / THETA ** (np.arange(0, ROPE_D, 2, dtype=np.float32) / ROPE_D)
    emb = np.arange(S, dtype=np.float32)[:, None] * freqs[None, :]  # [S, 48]
    cos48 = np.cos(emb).T.astype(np.float32)  # [48, S]
    sin48 = np.sin(emb).T.astype(np.float32)
    ccos = np.zeros((128, S), dtype=np.float32)
    ssin = np.zeros((128, S), dtype=np.float32)
    ccos[0:48] = cos48
    ccos[64:112] = cos48
    ssin[0:48] = sin48
    ssin[64:112] = sin48
    return (wq_t, wkv_t, wk_t, wv_t,
            ccos.astype(ml_dtypes.bfloat16), ssin.astype(ml_dtypes.bfloat16))


@with_exitstack
def mla_kernel(ctx: ExitStack, tc: tile.TileContext, xt_d, wq_d, wkv_d, wk_d, wv_d,
               ccos_d, ssin_d, out_d):
    nc = tc.nc

    pp_const = ctx.enter_context(tc.tile_pool(name="const", bufs=1))
    pp_x = ctx.enter_context(tc.tile_pool(name="xt", bufs=1))
    pp_ckv = ctx.enter_context(tc.tile_pool(name="ckv", bufs=1))
    pp_qk = ctx.enter_context(tc.tile_pool(name="qk", bufs=4))
    pp_w = ctx.enter_context(tc.tile_pool(name="wst", bufs=3))
    pp_rope = ctx.enter_context(tc.tile_pool(name="rope", bufs=2))
    pp_E = ctx.enter_context(tc.tile_pool(name="et", bufs=2))
    pp_v = ctx.enter_context(tc.tile_pool(name="vp", bufs=2))
    pp_o = ctx.enter_context(tc.tile_pool(name="ob", bufs=6))

    ps_a = ctx.enter_context(tc.tile_pool(name="ps_a", bufs=2, space="PSUM"))
    ps_qk = ctx.enter_context(tc.tile_pool(name="ps_qk", bufs=3, space="PSUM"))
    ps_pv = ctx.enter_context(tc.tile_pool(name="ps_pv", bufs=3, space="PSUM"))

    # --- constants ---
    ccos_t = pp_const.tile([128, S], BF16, tag="cct")
    ssin_t = pp_const.tile([128, S], BF16, tag="sst")

    xT2 = []
    wkv_t = []

    def fetch_x(c):
        xt = pp_x.tile([128, 2, S], BF16, tag=f"x{c}", name=f"x{c}")
        nc.sync.dma_start(
            xt[:], xt_d.ap()[c * 256:(c + 1) * 256, :]
            .rearrange("(c p) s -> p c s", p=128))
        xT2.append(xt)

    def fetch_wkv(m):
        wm = pp_w.tile([128, NK, 128], BF16, tag=f"wkv{m}", bufs=1, name=f"wkv{m}")
        nc.sync.dma_start(wm[:], wkv_d.ap()[m])
        wkv_t.append(wm)

    fetch_x(0)
    fetch_wkv(0)
    for c in range(1, 8):
        fetch_x(c)
    for m in range(1, NL):
        fetch_wkv(m)

    wq_t, wk_t, wv_t = {}, {}, {}

    def fetch_head(h):
        wq_t[h] = pp_w.tile([128, NK, 128], BF16, tag="wq", name=f"wq{h}")
        nc.sync.dma_start(wq_t[h][:], wq_d.ap()[h])
        wk_t[h] = pp_w.tile([128, NL, 128], BF16, tag="wk", name=f"wk{h}")
        nc.sync.dma_start(wk_t[h][:], wk_d.ap()[h])
        if h % 4 == 0:
            wv_t[h // 4] = pp_w.tile([128, NL, 512], BF16, tag="wv", bufs=2,
                                     name=f"wv{h // 4}")
            nc.sync.dma_start(wv_t[h // 4][:], wv_d.ap()[h // 4])

    fetch_head(0)
    nc.sync.dma_start(ccos_t[:], ccos_d.ap())
    nc.sync.dma_start(ssin_t[:], ssin_d.ap())

    def rope(t, eng, sfx):
        """In-place RoPE on a [128, S] head tile: rows [E(0:48)|n|O(64:112)|n]."""
        pc = pp_rope.tile([128, S], BF16, tag=f"pc{sfx}", name=f"pc{sfx}")
        pn = pp_rope.tile([128, S], BF16, tag=f"pn{sfx}", name=f"pn{sfx}")
        eng.tensor_tensor(pc[:], t[:], ccos_t[:], MULT)
        eng.tensor_tensor(pn[0:48, :], t[64:112, :], ssin_t[64:112, :], MULT)
        eng.tensor_tensor(pn[64:112, :], t[0:48, :], ssin_t[0:48, :], MULT)
        eng.tensor_tensor(t[0:48, :], pc[0:48, :], pn[0:48, :], SUB)
        eng.tensor_tensor(t[64:112, :], pc[64:112, :], pn[64:112, :], ADD)

    # --- ckv projection: c_kvT [latent, S] in 4 chunks ---
    ckv = [pp_ckv.tile([128, S], BF16, tag=f"ckv{j}", name=f"ckv{j}")
           for j in range(NL)]
    for m in range(NL):
        for n in range(2):
            ps = ps_a.tile([128, 512], F32, tag="pa")
            for k in range(NK):
                nc.tensor.matmul(ps[:], wkv_t[m][:, k],
                                 xT2[k // 2][:, k % 2, n * 512:(n + 1) * 512],
                                 start=(k == 0), stop=(k == NK - 1))
            nc.vector.tensor_copy(ckv[m][:, n * 512:(n + 1) * 512], ps[:])

    qT_t, kT_t, vt_t = {}, {}, {}

    def proj_k(h):
        kt = pp_qk.tile([128, S], BF16, tag="kt", name=f"kt{h}")
        kT_t[h] = kt
        for n in range(2):
            ps = ps_a.tile([128, 512], F32, tag="pa")
            for k in range(NL):
                nc.tensor.matmul(ps[:], wk_t[h][:, k],
                                 ckv[k][:, n * 512:(n + 1) * 512],
                                 start=(k == 0), stop=(k == NL - 1))
            nc.vector.tensor_copy(kt[:, n * 512:(n + 1) * 512], ps[:])
        rope(kt, nc.vector, "g")

    def proj_qv(h):
        qt = pp_qk.tile([128, S], BF16, tag="qt", name=f"qt{h}")
        qT_t[h] = qt
        for n in range(2):
            ps = ps_a.tile([128, 512], F32, tag="pa")
            for k in range(NK):
                nc.tensor.matmul(ps[:], wq_t[h][:, k],
                                 xT2[k // 2][:, k % 2, n * 512:(n + 1) * 512],
                                 start=(k == 0), stop=(k == NK - 1))
            nc.vector.tensor_copy(qt[:, n * 512:(n + 1) * 512], ps[:])
        rope(qt, nc.vector, "v")

        if h % 4 == 0:
            vt = pp_v.tile([128, NSC, 4, VW], BF16, tag="v", name=f"v{h // 4}")
            vt_t[h // 4] = vt
            nc.gpsimd.memset(vt[:, :, :, 128:129], 1.0)
            for sc in range(NSC):
                ps = ps_a.tile([128, 512], F32, tag="pa")
                for k in range(NL):
                    nc.tensor.matmul(ps[:],
                                     ckv[k][:, sc * 128:(sc + 1) * 128],
                                     wv_t[h // 4][:, k],
                                     start=(k == 0), stop=(k == NL - 1))
                nc.scalar.copy(
                    vt[:, sc, :, 0:128],
                    ps[:].rearrange("p (c d) -> p c d", c=4))

    def attn(h):
        hp, hs = h // 4, h % 4
        qt, kt, vt = qT_t[h], kT_t[h], vt_t[hp]
        # Et[kc] holds exp(scores) rows for k-chunk kc, cols q = c0..S
        Et = [pp_E.tile([128, S - 128 * kc], BF16, tag=f"e{kc}", name=f"e{kc}")
              for kc in range(NSC)]
        def scores(kc):
            c0 = 128 * kc
            for n in range(2):
                lo = max(n * 512, c0)
                hi = (n + 1) * 512
                if lo >= hi:
                    continue
                ps = ps_qk.tile([128, 512], F32, tag="qk")
                nc.tensor.matmul(ps[:, lo - 512 * n:512],
                                 kt[:, c0:c0 + 128], qt[:, lo:hi],
                                 start=True, stop=True)
                nc.scalar.activation(Et[kc][:, lo - c0:hi - c0],
                                     ps[:, lo - 512 * n:512], EXP, scale=SCALE)
                if lo == c0:
                    # causal mask of the diagonal block: zero where k > q
                    nc.gpsimd.affine_select(
                        out=Et[kc][:, 0:128], in_=Et[kc][:, 0:128],
                        compare_op=mybir.AluOpType.is_ge, fill=0.0,
                        base=0, pattern=[[1, 128]], channel_multiplier=-1)

        def pv(qc):
            ps = ps_pv.tile([128, 132], F32, tag="pv")
            for kc in range(qc + 1):
                nc.tensor.matmul(ps[:, 0:129],
                                 Et[kc][:, qc * 128 - kc * 128:(qc + 1) * 128 - kc * 128],
                                 vt[:, kc, hs, 0:129],
                                 start=(kc == 0), stop=(kc == qc))
            r = pp_o.tile([128, 1], F32, tag="r")
            nc.vector.reciprocal(r[:], ps[:, 128:129])
            ot = pp_o.tile([128, 128], F32, tag="ot")
            nc.vector.tensor_scalar(ot[:], ps[:, 0:128], r[:, 0:1], None, MULT)
            nc.sync.dma_start(
                out_d.ap()[qc * 128:(qc + 1) * 128, h * 128:(h + 1) * 128],
                ot[:])

        for kc in range(NSC):
            scores(kc)
        for qc in range(NSC):
            pv(qc)

    for h in range(H):
        if h + 1 < H:
            fetch_head(h + 1)
        proj_k(h)
        proj_qv(h)
        if h >= 1:
            attn(h - 1)
    attn(H - 1)


_CACHE = {}


def _build_nc(repeat=1):
    key = ("nc", repeat)
    if key in _CACHE:
        return _CACHE[key]
    nc = bacc.Bacc("TRN2", target_bir_lowering=False, debug=False, num_devices=B)
    xt_d = nc.dram_tensor("xt", [E, S], BF16, kind="ExternalInput")
    wq_d = nc.dram_tensor("wq", [H, 128, NK, 128], BF16, kind="ExternalInput")
    wkv_d = nc.dram_tensor("wkv", [NL, 128, NK, 128], BF16, kind="ExternalInput")
    wk_d = nc.dram_tensor("wk", [H, 128, NL, 128], BF16, kind="ExternalInput")
    wv_d = nc.dram_tensor("wv", [H // 4, 128, NL, 512], BF16, kind="ExternalInput")
    ccos_d = nc.dram_tensor("ccos", [128, S], BF16, kind="ExternalInput")
    ssin_d = nc.dram_tensor("ssin", [128, S], BF16, kind="ExternalInput")
    out_d = nc.dram_tensor("out", [S, E], F32, kind="ExternalOutput")

    with tile.TileContext(nc) as tc:
        for _ in range(repeat):
            mla_kernel(tc, xt_d, wq_d, wkv_d, wk_d, wv_d, ccos_d, ssin_d, out_d)
    nc.compile()
    _CACHE[key] = nc
    return nc


def kernel(x, Wq, Wkv_down, Wk_up, Wv_up, **run_kwargs):
    x = np.asarray(x, dtype=np.float32)
    wq_t, wkv_t, wk_t, wv_t, ccos, ssin = host_tensors(
        np.asarray(Wq, np.float32), np.asarray(Wkv_down, np.float32),
        np.asarray(Wk_up, np.float32), np.asarray(Wv_up, np.float32))
    nc = _build_nc()
    in_maps = [
        {"xt": np.ascontiguousarray(x[b].T.astype(ml_dtypes.bfloat16)),
         "wq": wq_t, "wkv": wkv_t, "wk": wk_t, "wv": wv_t,
         "ccos": ccos, "ssin": ssin}
        for b in range(B)
    ]
    res = run_bass_kernel_spmd(nc, in_maps, core_ids=list(range(B)), **run_kwargs)
    out = np.stack([res.results[b]["out"] for b in range(B)], axis=0)
    if run_kwargs:
        _CACHE["last_res"] = res
    return out
